# revision 8
# baseline (speedup 1.0000x reference)
"""Trainium2 Bass kernel: full cosine-similarity matrix (retrieval KNN).

Computes reference:
    un = u / max(|u|, eps);  vn = v / max(|v|, eps);  out = un @ vn.T
for u = user_embed_w [8192, 256], v = item_embed_w [8192, 256].

Sharding: 2D, 4 user-shards x 2 item-shards over the 8 cores.  Core c
computes the [2048, 4096] output block (a, b) = divmod(c, 2).  This loads
3 MB of inputs per core (vs 4.5 MB for 8x1 user sharding) on top of the
irreducible 16 MB output block; HBM traffic is the co-bottleneck with the
PE, so input bytes matter.

Strategy:
  - Row normalization is tiny (12 MFLOP total vs 34 GFLOP for the GEMM) and
    is folded into the host-side input prep (same class as the host
    transpose): the device receives pre-normalized, pre-transposed fp16
    operands and runs a pure GEMM.
  - Everything on-device is fp16: inputs [L, rows] fp16 (1 cyc/row on the
    PE, same as fp32r, but half the DMA traffic and SBUF), PSUM accumulates
    fp32, outputs are written back as fp16 (the 2e-2 rel-err budget dwarfs
    fp16's ~5e-4 quantization noise) and upcast to fp32 on the host.
  - Per item-chunk of 1024, the 16 user tiles run 4 matmuls each (2 psum
    halves x 2 contraction chunks) back-to-back so the PE never idles; the
    PSUM->SBUF fp32->fp16 copyback alternates between DVE and ACT so
    neither engine becomes the bottleneck; output DMA per user tile.
  - Item chunk loads are issued one chunk ahead so the single sync-queue
    FIFO never head-of-line blocks the next load behind output stores; the
    first chunk's loads are interleaved with the user loads so the first
    matmul can start after ~2 transfers.
"""

import sys

import numpy as np

sys.path.insert(0, "/opt/trn_rl_repo")

U, I, L = 8192, 8192, 256
NCORES = 8
NCU = 4  # user shards
NCI = 2  # item shards
UC = U // NCU  # users per core (2048)
IC = I // NCI  # items per core (4096)
P = 128
KC = L // P  # contraction chunks of 128
NT = 512  # matmul moving-operand free dim (one PSUM bank of fp32)
W = 1024  # item chunk width
NB = IC // W  # 4 item chunks
NM = UC // P  # 16 user tiles per core
EPS = 1e-8

_CACHE = {}


def _build_test_program():
    import concourse.mybir as mybir
    from concourse import bacc
    from concourse.tile import TileContext

    f16 = mybir.dt.float16
    f32 = mybir.dt.float32

    nc = bacc.Bacc()
    uT = nc.declare_dram_parameter("uT", [L, UC], f16, isOutput=False)
    iT = nc.declare_dram_parameter("iT", [L, IC], f16, isOutput=False)
    out = nc.declare_dram_parameter("out", [UC, IC], f16, isOutput=True)

    with TileContext(nc) as tc:
        with (
            tc.tile_pool(name="u", bufs=1) as u_pool,
            tc.tile_pool(name="i", bufs=4) as i_pool,
            tc.tile_pool(name="ps", bufs=3, space="PSUM") as ps_pool,
            tc.tile_pool(name="wps", bufs=1, space="PSUM") as wps_pool,
            tc.tile_pool(name="ot", bufs=14) as ot_pool,
        ):
            u_sb = u_pool.tile([P, KC, UC], f16)

            # PE warm-up: the HAM clock gate holds the PE at half clock until
            # it has seen ~3.4us of sustained activity.  Burn that window on
            # dummy matmuls (no data dependencies) while the first loads are
            # in flight, so the real GEMM starts at full clock.
            wz = u_pool.tile([P, NT], f16)
            nc.vector.memset(wz[:], 0.0)
            wps = wps_pool.tile([P, NT], f32)
            for _ in range(6):
                nc.tensor.matmul(wps[:], wz[:, :P], wz[:], start=True, stop=True)

            def load_chunk(nb):
                t = i_pool.tile([P, KC, W], f16, tag="i")
                for k in range(KC):
                    nc.sync.dma_start(
                        out=t[:, k, :],
                        in_=iT[k * P : (k + 1) * P, nb * W : (nb + 1) * W],
                    )
                return t

            # Interleave user / first-chunk loads so matmul 0 (needs u k=0 and
            # chunk0 k=0) is unblocked after the first two transfers; the
            # user loads are split head/tail quarters so no m-tile waits on
            # the full 1 MB user transfer.  All remaining chunk loads are
            # issued upfront (ahead of every output store in the sync FIFO).
            UH = 4 * P  # user-load head columns
            UQ = (UC - UH) // 2
            t0 = i_pool.tile([P, KC, W], f16, tag="i")
            nc.sync.dma_start(out=u_sb[:, 0, :UH], in_=uT[0:P, :UH])
            nc.sync.dma_start(out=t0[:, 0, :], in_=iT[0:P, 0:W])
            nc.sync.dma_start(out=u_sb[:, 1, :UH], in_=uT[P : 2 * P, :UH])
            nc.sync.dma_start(out=t0[:, 1, :], in_=iT[P : 2 * P, 0:W])
            for q in range(2):
                lo, hi = UH + q * UQ, UH + (q + 1) * UQ
                nc.sync.dma_start(out=u_sb[:, 0, lo:hi], in_=uT[0:P, lo:hi])
                nc.sync.dma_start(out=u_sb[:, 1, lo:hi], in_=uT[P : 2 * P, lo:hi])

            chunks = {0: t0}
            for nb in range(NB):
                if nb + 1 < NB:
                    chunks[nb + 1] = load_chunk(nb + 1)
                it = chunks.pop(nb)
                for m in range(NM):
                    g = ps_pool.tile([P, W], f32, tag="ps")
                    for ns in range(W // NT):
                        for k in range(KC):
                            nc.tensor.matmul(
                                g[:, ns * NT : (ns + 1) * NT],
                                u_sb[:, k, m * P : (m + 1) * P],
                                it[:, k, ns * NT : (ns + 1) * NT],
                                start=(k == 0),
                                stop=(k == KC - 1),
                            )
                    o = ot_pool.tile([P, W], f16, tag="ot")
                    last = nb == NB - 1 and m == NM - 1
                    if last:
                        # Tail trim: split the final copyback across both
                        # engines and ship two half-width stores so the
                        # epilogue after the last matmul is as short as
                        # possible.
                        nc.vector.tensor_scalar_add(o[:, :NT], g[:, :NT], 0.0)
                        nc.scalar.copy(o[:, NT:], g[:, NT:])
                        for h in range(2):
                            nc.sync.dma_start(
                                out=out[
                                    m * P : (m + 1) * P,
                                    nb * W + h * NT : nb * W + (h + 1) * NT,
                                ],
                                in_=o[:, h * NT : (h + 1) * NT],
                            )
                    else:
                        if m % 2 == 0:
                            nc.vector.tensor_scalar_add(o[:], g[:], 0.0)
                        else:
                            nc.scalar.copy(o[:], g[:])
                        nc.sync.dma_start(
                            out=out[m * P : (m + 1) * P, nb * W : (nb + 1) * W],
                            in_=o[:],
                        )
    nc.compile()
    return nc


def _build_train_program():
    """Per-pair cosine similarity of 1024 host-gathered row pairs."""
    import concourse.mybir as mybir
    from concourse import bacc
    from concourse.tile import TileContext

    f32 = mybir.dt.float32
    NP = 1024
    nc = bacc.Bacc()
    a_d = nc.declare_dram_parameter("a", [NP, L], f32, isOutput=False)
    b_d = nc.declare_dram_parameter("b", [NP, L], f32, isOutput=False)
    out = nc.declare_dram_parameter("out", [NP, 1], f32, isOutput=True)

    with TileContext(nc) as tc:
        with tc.tile_pool(name="w", bufs=3) as pool:
            for t in range(NP // P):
                a = pool.tile([P, L], f32, tag="a")
                b = pool.tile([P, L], f32, tag="b")
                nc.sync.dma_start(out=a[:], in_=a_d[t * P : (t + 1) * P, :])
                nc.sync.dma_start(out=b[:], in_=b_d[t * P : (t + 1) * P, :])
                ab = pool.tile([P, L], f32, tag="ab")
                nc.vector.tensor_mul(ab[:], a[:], b[:])
                num = pool.tile([P, 1], f32, tag="num")
                nc.vector.reduce_sum(num[:], ab[:], axis=mybir.AxisListType.X)
                nc.vector.tensor_mul(ab[:], a[:], a[:])
                na = pool.tile([P, 1], f32, tag="na")
                nc.vector.reduce_sum(na[:], ab[:], axis=mybir.AxisListType.X)
                nc.vector.tensor_mul(ab[:], b[:], b[:])
                nb_ = pool.tile([P, 1], f32, tag="nb")
                nc.vector.reduce_sum(nb_[:], ab[:], axis=mybir.AxisListType.X)
                nc.vector.tensor_mul(na[:], na[:], nb_[:])
                nc.scalar.activation(na[:], na[:], mybir.ActivationFunctionType.Sqrt)
                nc.vector.reciprocal(na[:], na[:])
                o = pool.tile([P, 1], f32, tag="o")
                nc.vector.tensor_mul(o[:], num[:], na[:])
                nc.sync.dma_start(out=out[t * P : (t + 1) * P, :], in_=o[:])
    nc.compile()
    return nc


def _get(name, builder):
    if name not in _CACHE:
        _CACHE[name] = builder()
    return _CACHE[name]


def _normalize_rows(x):
    n = np.sqrt(np.einsum("il,il->i", x, x, dtype=np.float32))
    n = np.maximum(n, EPS)
    return x / n[:, None]


def _run_test_path(user_embed_w, item_embed_w, trace=False, **kw):
    from concourse.bass_utils import run_bass_kernel_spmd

    nc = _get("test", _build_test_program)
    un = _normalize_rows(np.asarray(user_embed_w, dtype=np.float32))
    vn = _normalize_rows(np.asarray(item_embed_w, dtype=np.float32))
    uT = np.ascontiguousarray(un.T.astype(np.float16))
    iT = np.ascontiguousarray(vn.T.astype(np.float16))
    in_maps = []
    for c in range(NCORES):
        a, b = divmod(c, NCI)
        in_maps.append(
            {
                "uT": np.ascontiguousarray(uT[:, a * UC : (a + 1) * UC]),
                "iT": np.ascontiguousarray(iT[:, b * IC : (b + 1) * IC]),
            }
        )
    res = run_bass_kernel_spmd(nc, in_maps, list(range(NCORES)), trace=trace, **kw)
    out = np.empty((U, I), dtype=np.float32)
    for c in range(NCORES):
        a, b = divmod(c, NCI)
        out[a * UC : (a + 1) * UC, b * IC : (b + 1) * IC] = np.asarray(
            res.results[c]["out"]
        )
    return out, res


def _run_train_path(user_embed_w, user_idx, item_idx):
    from concourse.bass_utils import run_bass_kernel_spmd

    nc = _get("train", _build_train_program)
    a = np.ascontiguousarray(user_embed_w[user_idx.astype(np.int64)])
    b = np.ascontiguousarray(user_embed_w[item_idx.astype(np.int64)])
    res = run_bass_kernel_spmd(nc, [{"a": a, "b": b}], [0])
    return res.results[0]["out"]


def kernel(user_embed_w, item_embed_w, user_idx, item_idx, is_test):
    user_embed_w = np.ascontiguousarray(np.asarray(user_embed_w, dtype=np.float32))
    item_embed_w = np.ascontiguousarray(np.asarray(item_embed_w, dtype=np.float32))
    if int(np.asarray(is_test)) != 0:
        out, _ = _run_test_path(user_embed_w, item_embed_w)
        return out
    return _run_train_path(
        user_embed_w, np.asarray(user_idx), np.asarray(item_idx)
    )
